# revision 25
# baseline (speedup 1.0000x reference)
"""Trainium2 Bass kernel for nn_Attention_32650341384246.

Full attention layer: qkv proj + per-head RMSNorm(q,k) + RoPE + softmax
attention (non-causal) + out proj.  B=2, S=2048, D=1024, H=16, DH=64.

Sharding: 8 cores; core c handles batch c//4, heads [4*(c%4), 4*(c%4)+4)
(data parallel over batch x tensor parallel over heads).  Each core
computes a partial [S, D] output (its heads @ Wout row-slice); the host
sums the 4 partials per batch and adds the (folded) biases.

Device design (per core):
  - x fed pre-transposed+bf16 as xT [128, 8, 2048]  (p + 128*a = model dim)
  - qkv proj emits qT/kT head-major [128 (2 heads x 64), S] directly
    (lhsT = W slice, rhs = xT slice) and v s-major [s, 4*64].
  - RMSNorm in head-major layout: sum(x^2) over d via ones-block matmul
    (f32r), rsqrt = Exp(-0.5*Ln(mean+eps)) on ACT (same table set as the
    softmax Exp -> zero table switches), partition-broadcast via ones
    matmul.
  - RoPE as q_rot = cosT*u + sinT'*swap(u); swap = adjacent-partition
    permutation matmul; cos/sin tables host-built from `pos` with
    q_scale/k_scale folded in; 1/sqrt(dh) folded into the exp scale.
  - scores^T [k, q] bf16 matmuls (K=64, tile_position row groups),
    PSUM [128, 1024] per head, staggered h0/h1 so ACT exp pipelines
    against PE; exp reads PSUM, writes bf16.
  - AV via lhsT = [v | ones] bf16 (M=65): row 64 accumulates sumexp.
  - normalize: gather 4 sumexp rows -> one DVE reciprocal [4, 512],
    select-matrix matmul broadcasts reciprocal rows across partitions.
  - out proj: lhsT = v_mixT bf16, rhs = Wout row-slice bf16.
Heavy matmuls are bf16 (fp32 PSUM accumulate); small helper matmuls
(sumsq / broadcasts / swap) stay float32r.
"""
import sys, os

sys.path.insert(0, "/opt/trn_rl_repo")

import numpy as np
from contextlib import ExitStack

import ml_dtypes
import concourse.bass as bass
import concourse.mybir as mybir
import concourse.tile as tile
from concourse import bacc
from concourse import bass_utils

F32 = mybir.dt.float32
F32R = mybir.dt.float32r
BF16 = mybir.dt.bfloat16
AF = mybir.ActivationFunctionType

B, S, DM, H, DH = 2, 2048, 1024, 16, 64
NC = 8
HPC = H // 4          # 4 heads per core
HD = HPC * DH         # 256
NDT = DM // 128       # 8 model-dim tiles
THETA, EPS = 10000.0, 1e-6

LAST_RESULTS = None   # BassKernelResults of the most recent device run
_CACHED = {}


def build_program(exp_scale: float, shared_tables: bool):
    nc = bacc.Bacc("TRN2", target_bir_lowering=False, debug=False)

    xT_d = nc.dram_tensor("xT", [128, NDT, S], BF16, kind="ExternalInput")
    w_d = nc.dram_tensor("w_all", [128, NDT, 3 * HD], BF16, kind="ExternalInput")
    wout_d = nc.dram_tensor("wout", [128, 2, DM], BF16, kind="ExternalInput")
    bq_d = nc.dram_tensor("bq", [128, 2], F32, kind="ExternalInput")
    bk_d = nc.dram_tensor("bk", [128, 2], F32, kind="ExternalInput")
    cosk_d = nc.dram_tensor("cos_k", [128, S], F32, kind="ExternalInput")
    sink_d = nc.dram_tensor("sin_k", [128, S], F32, kind="ExternalInput")
    if not shared_tables:
        cosq_d = nc.dram_tensor("cos_q", [128, S], F32, kind="ExternalInput")
        sinq_d = nc.dram_tensor("sin_q", [128, S], F32, kind="ExternalInput")
    P_d = nc.dram_tensor("Pswap", [128, 128], F32R, kind="ExternalInput")
    ob_d = nc.dram_tensor("onesblk", [128, 2], F32R, kind="ExternalInput")
    o2_d = nc.dram_tensor("ones2blk", [2, 128], F32R, kind="ExternalInput")
    sel_d = nc.dram_tensor("sel", [128, 2, 128], F32R, kind="ExternalInput")
    out_d = nc.dram_tensor("outp", [S, DM], BF16, kind="ExternalOutput")

    with tile.TileContext(nc) as tc, ExitStack() as ctx, \
            nc.allow_low_precision(reason="fp32r/bf16 matmul inputs"):
        singles = ctx.enter_context(tc.tile_pool(name="singles", bufs=1))
        tmp = ctx.enter_context(tc.tile_pool(name="tmp", bufs=2))
        expp = ctx.enter_context(tc.tile_pool(name="expp", bufs=4))
        outp = ctx.enter_context(tc.tile_pool(name="outp", bufs=4))

        # --- first-needed loads up front; per-dt tiles so Tile's
        # per-tile RAW tracking doesn't serialize readers behind all DMAs ---
        w_dt = [singles.tile([128, 3 * HD], BF16, name=f"w{dt}") for dt in range(NDT)]
        x_dt = [singles.tile([128, S], BF16, name=f"x{dt}") for dt in range(NDT)]
        for dt in range(NDT):
            nc.sync.dma_start(out=w_dt[dt], in_=w_d.ap()[:, dt, :])
            nc.sync.dma_start(out=x_dt[dt], in_=xT_d.ap()[:, dt, :])

        wout = singles.tile([128, 2, DM], BF16)
        nc.sync.dma_start(out=wout, in_=wout_d.ap())
        bq = singles.tile([128, 2], F32)
        nc.sync.dma_start(out=bq, in_=bq_d.ap())
        bk = singles.tile([128, 2], F32)
        nc.sync.dma_start(out=bk, in_=bk_d.ap())
        cos_k = singles.tile([128, S], F32)
        nc.sync.dma_start(out=cos_k, in_=cosk_d.ap())
        sin_k = singles.tile([128, S], F32)
        nc.sync.dma_start(out=sin_k, in_=sink_d.ap())
        if shared_tables:
            cos_q, sin_q = cos_k, sin_k
        else:
            cos_q = singles.tile([128, S], F32)
            nc.sync.dma_start(out=cos_q, in_=cosq_d.ap())
            sin_q = singles.tile([128, S], F32)
            nc.sync.dma_start(out=sin_q, in_=sinq_d.ap())
        Pm = singles.tile([128, 128], F32R)
        nc.sync.dma_start(out=Pm, in_=P_d.ap())
        onesblk = singles.tile([128, 2], F32R)
        nc.sync.dma_start(out=onesblk, in_=ob_d.ap())
        ones2blk = singles.tile([2, 128], F32R)
        nc.sync.dma_start(out=ones2blk, in_=o2_d.ap())
        sel = singles.tile([128, 2, 128], F32R)
        nc.sync.dma_start(out=sel, in_=sel_d.ap())
        eps_t = singles.tile([128, 1], F32)
        nc.vector.memset(eps_t, EPS)

        qt = [[singles.tile([128, 512], BF16, name=f"qt{t}_{sc}")
               for sc in range(4)] for t in range(2)]
        kt_ = [[singles.tile([128, 512], BF16, name=f"kt{t}_{sc}")
                for sc in range(4)] for t in range(2)]
        vhat = [singles.tile([128, 4, HPC, 65], BF16, name=f"vhat{sc}")
                for sc in range(4)]
        for sc in range(4):
            nc.vector.memset(vhat[sc][:, :, :, 64:65], 1.0)
        vmix = [[singles.tile([128, 1024], BF16, name=f"vmix{t}_{qh}")
                 for qh in range(2)] for t in range(2)]

        # ---------------- phase 1: qkv + rmsnorm + rope ----------------
        # section-major order [k0, q0, k1, q1, v]: pair-0 attention deps
        # complete early; v matmuls keep PE dense while the rope tail drains.
        # All Ln ops forced before all Exp ops -> exactly 2 ACT table loads.
        ln_insts, exp_insts = [], []
        with tc.tile_pool(name="ps1", bufs=1, space="PSUM") as ps1:
            sections = (
                    ("k", 0, bk, cos_k, sin_k, kt_),
                    ("v", 0, None, None, None, None),
                    ("q", 0, bq, cos_q, sin_q, qt),
                    ("v", 1, None, None, None, None),
                    ("k", 1, bk, cos_k, sin_k, kt_),
                    ("v", 2, None, None, None, None),
                    ("q", 1, bq, cos_q, sin_q, qt),
                    ("v", 3, None, None, None, None))
            for which, t, bias, cosT, sinT, dest in sections:
                if which == "v":
                    sc = t
                    for st in range(4):
                        pv = ps1.tile([128, HD], F32, tag="pv", bufs=2,
                                      name=f"pv{sc}_{st}")
                        for dt in range(NDT):
                            nc.tensor.matmul(
                                pv[:, :],
                                x_dt[dt][:, sc * 512 + st * 128: sc * 512 + (st + 1) * 128],
                                w_dt[dt][:, 2 * HD:3 * HD],
                                start=(dt == 0), stop=(dt == NDT - 1))
                        nc.vector.tensor_copy(vhat[sc][:, st, :, 0:64],
                                              pv[:, :].rearrange("p (h d) -> p h d", h=HPC))
                    continue
                off = 0 if which == "q" else HD
                for sc in range(4):       # s-chunks of 512
                    s0 = sc * 512
                    pq = ps1.tile([128, 512], F32, tag="pq", bufs=2,
                                  name=f"pq{which}{t}_{sc}")
                    for dt in range(NDT):
                        nc.tensor.matmul(
                            pq[:, :],
                            w_dt[dt][:, off + t * 128: off + (t + 1) * 128],
                            x_dt[dt][:, s0:s0 + 512],
                            start=(dt == 0), stop=(dt == NDT - 1))
                    tt = tmp.tile([128, 512], F32, tag="tt", bufs=5, name=f"tt{which}{t}_{sc}")
                    nc.scalar.activation(tt[:, :], pq[:, :], AF.Identity,
                                         bias=bias[:, t:t + 1], scale=1.0)
                    sq = tmp.tile([128, 512], F32R, tag="sq", bufs=3, name=f"sq{which}{t}_{sc}")
                    nc.scalar.activation(sq[:, :], tt[:, :], AF.Square)
                    pss = ps1.tile([2, 512], F32, tag="pss", bufs=2,
                                   name=f"pss{which}{t}_{sc}")
                    nc.tensor.matmul(pss[:, :], onesblk[:, :], sq[:, :],
                                     start=True, stop=True)
                    # rsqrt(mean+eps) = Exp(-0.5*Ln(mean+eps)); both funcs in
                    # the natural_log_exp table set shared with softmax Exp
                    lns = tmp.tile([2, 512], F32, tag="lns", bufs=5, name=f"lns{which}{t}_{sc}")
                    li = nc.scalar.activation(lns[:, :], pss[:, :], AF.Ln,
                                              bias=eps_t[0:2, :], scale=1.0 / DH)
                    ln_insts.append(li)
                    rs = tmp.tile([2, 512], F32R, tag="rs", bufs=3, name=f"rs{which}{t}_{sc}")
                    ei = nc.scalar.activation(rs[:, :], lns[:, :], AF.Exp,
                                              scale=-0.5)
                    exp_insts.append(ei)
                    pb = ps1.tile([128, 512], F32, tag="pb",
                                  name=f"pb{which}{t}_{sc}")
                    nc.tensor.matmul(pb[:, :], ones2blk[:, :], rs[:, :],
                                     start=True, stop=True)
                    u = tmp.tile([128, 512], F32R, tag="u", bufs=3, name=f"u{which}{t}_{sc}")
                    nc.vector.tensor_mul(u[:, :], tt[:, :], pb[:, :])
                    psw = ps1.tile([128, 512], F32, tag="psw",
                                   name=f"psw{which}{t}_{sc}")
                    nc.tensor.matmul(psw[:, :], Pm[:, :], u[:, :],
                                     start=True, stop=True)
                    t1 = tmp.tile([128, 512], F32, tag="t1", bufs=3, name=f"t1{which}{t}_{sc}")
                    nc.vector.tensor_mul(t1[:, :], u[:, :].bitcast(F32),
                                         cosT[:, s0:s0 + 512])
                    t2 = tmp.tile([128, 512], F32, tag="t2", bufs=3, name=f"t2{which}{t}_{sc}")
                    nc.vector.tensor_mul(t2[:, :], psw[:, :], sinT[:, s0:s0 + 512])
                    nc.vector.tensor_add(dest[t][sc][:, :], t1[:, :], t2[:, :])

        for g in range(4):            # per-section groups (k0, q0, k1, q1)
            last_ln = ln_insts[g * 4 + 3]
            for ei in exp_insts[g * 4: g * 4 + 4]:
                tile.add_dep_helper(ei.ins, last_ln.ins, sync=False,
                                    reason="group ACT ln before exp (table sets)")

        # ---------------- phase 2: attention ----------------
        with tc.tile_pool(name="ps2", bufs=1, space="PSUM") as ps2:
            for pair in range(2):
                for qh in range(2):
                    q0 = qh * 1024
                    ps_sc = [ps2.tile([128, 1024], F32, tag=f"sc{h}",
                                      name=f"sc{pair}{qh}{h}") for h in range(2)]
                    ps_av = [[ps2.tile([65, 512], F32, tag=f"av{h}{qc}",
                                       name=f"av{pair}{qh}{h}{qc}")
                              for qc in range(2)] for h in range(2)]
                    for kt in range(16):
                        # scores: alternate heads so adjacent PE matmuls sit on
                        # different row groups (tile_position) and PSUM banks
                        for qc in range(2):
                            for h in range(2):
                                nc.tensor.matmul(
                                    ps_sc[h][:, qc * 512:(qc + 1) * 512],
                                    kt_[pair][kt // 4][h * 64:(h + 1) * 64,
                                                       (kt % 4) * 128:(kt % 4 + 1) * 128],
                                    qt[pair][qh * 2 + qc][h * 64:(h + 1) * 64, :],
                                    start=True, stop=True,
                                    tile_position=(h * 64, 0))
                        es = []
                        for h in range(2):
                            e = expp.tile([128, 1024], BF16, tag=f"e{h}",
                                          name=f"e{pair}{qh}{h}_{kt}")
                            nc.scalar.activation(e[:, :], ps_sc[h][:, :], AF.Exp,
                                                 scale=exp_scale)
                            es.append(e)
                        for h in range(2):
                            head = 2 * pair + h
                            for qc in range(2):
                                nc.tensor.matmul(
                                    ps_av[h][qc][:, :],
                                    vhat[kt // 4][:, kt % 4, head, :],
                                    es[h][:, qc * 512:(qc + 1) * 512],
                                    start=(kt == 0), stop=(kt == 15),
                                    skip_group_check=True)
                    # normalize: batch the 4 sumexp rows -> one reciprocal
                    # (rows live at 32-aligned partitions; rest memset to 1.0
                    # so the reciprocal stays finite and sel rows zero them)
                    se = tmp.tile([128, 512], F32, tag="se", name=f"se{pair}{qh}")
                    nc.vector.memset(se, 1.0)
                    for h in range(2):
                        for qc in range(2):
                            r0 = 32 * (2 * h + qc)
                            nc.vector.tensor_copy(se[r0:r0 + 1, :],
                                                  ps_av[h][qc][64:65, :])
                    recip4 = tmp.tile([128, 512], F32R, tag="recip4",
                                      name=f"rc{pair}{qh}")
                    from concourse.dve_ops import (RECIP_APPROX_FAST_CONSTS,
                                                   RECIPROCAL_APPROX_FAST)
                    _c = RECIP_APPROX_FAST_CONSTS
                    nc.vector._custom_dve(RECIPROCAL_APPROX_FAST,
                                          out=recip4[:, :], in0=se[:, :],
                                          s0=_c["s0"], s1=_c["s1"],
                                          imm2=_c["imm2"])
                    for qc in range(2):
                        col = q0 + qc * 512
                        avs2 = tmp.tile([128, 512], F32, tag="avs2",
                                        name=f"avs{pair}{qh}{qc}")
                        for h in range(2):
                            nc.vector.tensor_copy(avs2[h * 64:(h + 1) * 64, :],
                                                  ps_av[h][qc][0:64, :])
                        pb2 = ps2.tile([128, 512], F32, tag=f"av0{qc}",
                                       name=f"nb{pair}{qh}{qc}")
                        nc.tensor.matmul(pb2[:, :], sel[:, qc, :], recip4[:, :],
                                         start=True, stop=True)
                        nc.vector.tensor_mul(
                            vmix[pair][qh][:, qc * 512:(qc + 1) * 512],
                            avs2[:, :], pb2[:, :])

        # ---------------- phase 3: out proj ----------------
        with tc.tile_pool(name="ps3", bufs=1, space="PSUM") as ps3:
            for st in range(16):
                for n in range(2):
                    po = ps3.tile([128, 512], F32, tag="po", bufs=3,
                                  name=f"po{st}_{n}")
                    for t in range(2):
                        nc.tensor.matmul(
                            po[:, :],
                            vmix[t][st // 8][:, (st % 8) * 128:(st % 8 + 1) * 128],
                            wout[:, t, n * 512:(n + 1) * 512],
                            start=(t == 0), stop=(t == 1))
                    o = outp.tile([128, 512], BF16, tag="o", name=f"o{st}_{n}")
                    if (st * 2 + n) % 2 == 0:
                        nc.scalar.activation(o[:, :], po[:, :], AF.Copy)
                    else:
                        nc.vector.tensor_copy(o[:, :], po[:, :])
                    nc.sync.dma_start(
                        out=out_d.ap()[st * 128:(st + 1) * 128,
                                       n * 512:(n + 1) * 512],
                        in_=o[:, :])

    nc.compile()
    return nc


def host_prep(x, pos, Wqkv, bqkv, Wout, bout, q_scale, k_scale):
    """Build per-core input maps + shared-table decision."""
    x = np.asarray(x, dtype=np.float32)
    pos = np.asarray(pos, dtype=np.float32).reshape(-1)
    Wqkv = np.asarray(Wqkv, dtype=np.float32)
    bqkv = np.asarray(bqkv, dtype=np.float32)
    Wout = np.asarray(Wout, dtype=np.float32)
    q_scale = np.asarray(q_scale, dtype=np.float32)
    k_scale = np.asarray(k_scale, dtype=np.float32)

    shared = bool(np.array_equal(q_scale, k_scale))
    exp_scale = (1.0 / np.sqrt(DH)) if shared else 1.0

    # rope base tables [128, S]
    i_of_p = (np.arange(128) % 64) // 2            # pair index
    sign = np.where(np.arange(128) % 2 == 0, 1.0, -1.0)
    omega = THETA ** (-np.arange(0, DH, 2, dtype=np.float64) / DH)  # [32]
    ang = pos[None, :].astype(np.float64) * omega[:, None]          # [32, S]
    cosb = np.cos(ang)[i_of_p, :]                  # [128, S]
    sinb = np.sin(ang)[i_of_p, :] * sign[:, None]

    def tables(scale_vec, extra):
        sv = np.tile(scale_vec, 2)                 # [128]
        svx = np.tile(scale_vec[np.arange(64) ^ 1], 2)
        cosT = (cosb * sv[:, None] * extra).astype(np.float32)
        sinT = (sinb * svx[:, None] * extra).astype(np.float32)
        return np.ascontiguousarray(cosT), np.ascontiguousarray(sinT)

    cos_k, sin_k = tables(k_scale, 1.0)
    if not shared:
        cos_q, sin_q = tables(q_scale, 1.0 / np.sqrt(DH))

    Pm = np.zeros((128, 128), dtype=np.float32)
    Pm[np.arange(128), np.arange(128) ^ 1] = 1.0
    onesblk = np.zeros((128, 2), dtype=np.float32)
    onesblk[0:64, 0] = 1.0
    onesblk[64:128, 1] = 1.0
    ones2blk = np.zeros((2, 128), dtype=np.float32)
    ones2blk[0, 0:64] = 1.0
    ones2blk[1, 64:128] = 1.0
    # sel[qc]: [4, 128] selecting reciprocal row (h, qc) for partitions h*64..
    sel = np.zeros((128, 2, 128), dtype=np.float32)
    for qc in range(2):
        for h in range(2):
            sel[32 * (2 * h + qc), qc, h * 64:(h + 1) * 64] = 1.0

    bf = ml_dtypes.bfloat16
    in_maps = []
    for c in range(NC):
        b, g = c // 4, c % 4
        xT = np.ascontiguousarray(
            x[b].T.reshape(NDT, 128, S).transpose(1, 0, 2)).astype(bf)
        wq = Wqkv[:, g * HD:(g + 1) * HD]
        wk = Wqkv[:, DM + g * HD: DM + (g + 1) * HD]
        wv = Wqkv[:, 2 * DM + g * HD: 2 * DM + (g + 1) * HD]
        w_all = np.ascontiguousarray(
            np.concatenate([wq, wk, wv], axis=1)
            .reshape(NDT, 128, 3 * HD).transpose(1, 0, 2)).astype(bf)
        wo = np.ascontiguousarray(
            Wout[g * HD:(g + 1) * HD, :]
            .reshape(2, 128, DM).transpose(1, 0, 2)).astype(bf)
        bqs = np.ascontiguousarray(
            bqkv[g * HD:(g + 1) * HD].reshape(2, 128).T)         # [128, 2]
        bks = np.ascontiguousarray(
            bqkv[DM + g * HD: DM + (g + 1) * HD].reshape(2, 128).T)
        m = {"xT": xT, "w_all": w_all, "wout": wo, "bq": bqs, "bk": bks,
             "cos_k": cos_k, "sin_k": sin_k, "Pswap": Pm, "onesblk": onesblk,
             "ones2blk": ones2blk, "sel": sel}
        if not shared:
            m["cos_q"] = cos_q
            m["sin_q"] = sin_q
        in_maps.append(m)

    bias_row = (bqkv[2 * DM:] @ Wout + np.asarray(bout, dtype=np.float32)) \
        .astype(np.float32)                                       # [1024]
    return in_maps, shared, float(exp_scale), bias_row


def _install_ntff_shim():
    """Make trace=True usable: this image lacks antenv.axon_hooks; recreate
    it against the baked libaxon_pjrt.so C ABI (no-op if already present)."""
    try:
        from antenv.axon_hooks import get_axon_ntff_profile_hook  # noqa: F401
        return
    except ImportError:
        pass
    try:
        import types, ctypes, contextlib
        import antenv
        lib = ctypes.CDLL("/opt/axon/libaxon_pjrt.so")
        if not hasattr(lib, "axon_start_nrt_profile"):
            raise OSError("no profile symbols")
        lib.axon_start_nrt_profile.argtypes = [ctypes.POINTER(ctypes.c_int64),
                                               ctypes.c_size_t]
        lib.axon_start_nrt_profile.restype = ctypes.c_int64
        lib.axon_stop_nrt_profile.argtypes = [ctypes.c_char_p]
        lib.axon_stop_nrt_profile.restype = ctypes.c_int64

        @contextlib.contextmanager
        def _hook(output_dir, device_ids):
            import jax
            jax.devices()
            if device_ids:
                ids = (ctypes.c_int64 * len(device_ids))(*device_ids)
                rc = lib.axon_start_nrt_profile(ids, len(device_ids))
            else:
                rc = lib.axon_start_nrt_profile(None, 0)
            if rc != 0:
                raise RuntimeError(f"axon_start_nrt_profile rc={rc}")
            try:
                yield
            finally:
                lib.axon_stop_nrt_profile(str(output_dir).encode())

        mod = types.ModuleType("antenv.axon_hooks")
        mod.get_axon_ntff_profile_hook = lambda: _hook
        mod.set_axon_ntff_profile_hook = lambda h: None
        sys.modules["antenv.axon_hooks"] = mod
        antenv.axon_hooks = mod
    except Exception:
        os.environ["BASS_NEVER_TRACE"] = "1"   # degrade: run untraced


def kernel(x, pos, Wqkv, bqkv, Wout, bout, q_scale, k_scale):
    global LAST_RESULTS
    if os.environ.get("BASS_TRACE"):
        _install_ntff_shim()
    in_maps, shared, exp_scale, bias_row = host_prep(
        x, pos, Wqkv, bqkv, Wout, bout, q_scale, k_scale)

    key = (shared, round(exp_scale, 9))
    if key not in _CACHED:
        _CACHED[key] = build_program(exp_scale, shared)
    nc = _CACHED[key]

    res = bass_utils.run_bass_kernel_spmd(
        nc, in_maps, list(range(NC)),
        trace=bool(os.environ.get("BASS_TRACE")))
    LAST_RESULTS = res

    out = np.empty((B, S, DM), dtype=np.float32)
    for b in range(B):
        acc = bias_row[None, :].astype(np.float32).repeat(S, axis=0)
        for g in range(4):
            acc = acc + res.results[b * 4 + g]["outp"].astype(np.float32)
        out[b] = acc
    return out


# revision 26
# speedup vs baseline: 1.0206x; 1.0206x over previous
"""Trainium2 Bass kernel for nn_Attention_32650341384246.

Full attention layer: qkv proj + per-head RMSNorm(q,k) + RoPE + softmax
attention (non-causal) + out proj.  B=2, S=2048, D=1024, H=16, DH=64.

Sharding: 8 cores; core c handles batch c//4, heads [4*(c%4), 4*(c%4)+4)
(data parallel over batch x tensor parallel over heads).  Each core
computes a partial [S, D] output (its heads @ Wout row-slice); the host
sums the 4 partials per batch and adds the (folded) biases.

Device design (per core):
  - x fed pre-transposed+bf16 as xT [128, 8, 2048]  (p + 128*a = model dim)
  - qkv proj emits qT/kT head-major [128 (2 heads x 64), S] directly
    (lhsT = W slice, rhs = xT slice) and v s-major [s, 4*64].
  - RMSNorm in head-major layout: sum(x^2) over d via ones-block matmul
    (f32r), rsqrt = Exp(-0.5*Ln(mean+eps)) on ACT (same table set as the
    softmax Exp -> zero table switches), partition-broadcast via ones
    matmul.
  - RoPE as q_rot = cosT*u + sinT'*swap(u); swap = adjacent-partition
    permutation matmul; cos/sin tables host-built from `pos` with
    q_scale/k_scale folded in; 1/sqrt(dh) folded into the exp scale.
  - scores^T [k, q] bf16 matmuls (K=64, tile_position row groups),
    PSUM [128, 1024] per head, staggered h0/h1 so ACT exp pipelines
    against PE; exp reads PSUM, writes bf16.
  - AV via lhsT = [v | ones] bf16 (M=65): row 64 accumulates sumexp.
  - normalize: gather 4 sumexp rows -> one DVE reciprocal [4, 512],
    select-matrix matmul broadcasts reciprocal rows across partitions.
  - out proj: lhsT = v_mixT bf16, rhs = Wout row-slice bf16.
Heavy matmuls are bf16 (fp32 PSUM accumulate); small helper matmuls
(sumsq / broadcasts / swap) stay float32r.
"""
import sys, os

sys.path.insert(0, "/opt/trn_rl_repo")

import numpy as np
from contextlib import ExitStack

import ml_dtypes
import concourse.bass as bass
import concourse.mybir as mybir
import concourse.tile as tile
from concourse import bacc
from concourse import bass_utils

F32 = mybir.dt.float32
F32R = mybir.dt.float32r
BF16 = mybir.dt.bfloat16
AF = mybir.ActivationFunctionType

B, S, DM, H, DH = 2, 2048, 1024, 16, 64
NC = 8
HPC = H // 4          # 4 heads per core
HD = HPC * DH         # 256
NDT = DM // 128       # 8 model-dim tiles
THETA, EPS = 10000.0, 1e-6

LAST_RESULTS = None   # BassKernelResults of the most recent device run
_CACHED = {}


def build_program(exp_scale: float, shared_tables: bool):
    nc = bacc.Bacc("TRN2", target_bir_lowering=False, debug=False)

    xT_d = nc.dram_tensor("xT", [128, NDT, S], BF16, kind="ExternalInput")
    w_d = nc.dram_tensor("w_all", [128, NDT, 3 * HD], BF16, kind="ExternalInput")
    wout_d = nc.dram_tensor("wout", [128, 2, DM], BF16, kind="ExternalInput")
    bq_d = nc.dram_tensor("bq", [128, 2], F32, kind="ExternalInput")
    bk_d = nc.dram_tensor("bk", [128, 2], F32, kind="ExternalInput")
    cosk_d = nc.dram_tensor("cos_k", [128, S], F32, kind="ExternalInput")
    sink_d = nc.dram_tensor("sin_k", [128, S], F32, kind="ExternalInput")
    if not shared_tables:
        cosq_d = nc.dram_tensor("cos_q", [128, S], F32, kind="ExternalInput")
        sinq_d = nc.dram_tensor("sin_q", [128, S], F32, kind="ExternalInput")
    P_d = nc.dram_tensor("Pswap", [128, 128], F32R, kind="ExternalInput")
    ob_d = nc.dram_tensor("onesblk", [128, 2], F32R, kind="ExternalInput")
    o2_d = nc.dram_tensor("ones2blk", [2, 128], F32R, kind="ExternalInput")
    sel_d = nc.dram_tensor("sel", [128, 2, 128], F32R, kind="ExternalInput")
    out_d = nc.dram_tensor("outp", [S, DM], BF16, kind="ExternalOutput")

    with tile.TileContext(nc) as tc, ExitStack() as ctx, \
            nc.allow_low_precision(reason="fp32r/bf16 matmul inputs"):
        singles = ctx.enter_context(tc.tile_pool(name="singles", bufs=1))
        tmp = ctx.enter_context(tc.tile_pool(name="tmp", bufs=2))
        expp = ctx.enter_context(tc.tile_pool(name="expp", bufs=4))
        outp = ctx.enter_context(tc.tile_pool(name="outp", bufs=4))

        # --- first-needed loads up front; per-dt tiles so Tile's
        # per-tile RAW tracking doesn't serialize readers behind all DMAs ---
        w_dt = [singles.tile([128, 3 * HD], BF16, name=f"w{dt}") for dt in range(NDT)]
        x_dt = [singles.tile([128, S], BF16, name=f"x{dt}") for dt in range(NDT)]
        for dt in range(NDT):
            nc.sync.dma_start(out=w_dt[dt], in_=w_d.ap()[:, dt, :])
            nc.sync.dma_start(out=x_dt[dt], in_=xT_d.ap()[:, dt, :])

        wout = singles.tile([128, 2, DM], BF16)
        nc.sync.dma_start(out=wout, in_=wout_d.ap())
        bq = singles.tile([128, 2], F32)
        nc.sync.dma_start(out=bq, in_=bq_d.ap())
        bk = singles.tile([128, 2], F32)
        nc.sync.dma_start(out=bk, in_=bk_d.ap())
        cos_k = singles.tile([128, S], F32)
        nc.sync.dma_start(out=cos_k, in_=cosk_d.ap())
        sin_k = singles.tile([128, S], F32)
        nc.sync.dma_start(out=sin_k, in_=sink_d.ap())
        if shared_tables:
            cos_q, sin_q = cos_k, sin_k
        else:
            cos_q = singles.tile([128, S], F32)
            nc.sync.dma_start(out=cos_q, in_=cosq_d.ap())
            sin_q = singles.tile([128, S], F32)
            nc.sync.dma_start(out=sin_q, in_=sinq_d.ap())
        Pm = singles.tile([128, 128], F32R)
        nc.sync.dma_start(out=Pm, in_=P_d.ap())
        onesblk = singles.tile([128, 2], F32R)
        nc.sync.dma_start(out=onesblk, in_=ob_d.ap())
        ones2blk = singles.tile([2, 128], F32R)
        nc.sync.dma_start(out=ones2blk, in_=o2_d.ap())
        sel = singles.tile([128, 2, 128], F32R)
        nc.sync.dma_start(out=sel, in_=sel_d.ap())
        eps_t = singles.tile([128, 1], F32)
        nc.vector.memset(eps_t, EPS)

        qt = [[singles.tile([128, 512], BF16, name=f"qt{t}_{sc}")
               for sc in range(4)] for t in range(2)]
        kt_ = [[singles.tile([128, 512], BF16, name=f"kt{t}_{sc}")
                for sc in range(4)] for t in range(2)]
        vhat = [singles.tile([128, 4, HPC, 65], BF16, name=f"vhat{sc}")
                for sc in range(4)]
        for sc in range(4):
            nc.vector.memset(vhat[sc][:, :, :, 64:65], 1.0)
        vmix = [[singles.tile([128, 1024], BF16, name=f"vmix{t}_{qh}")
                 for qh in range(2)] for t in range(2)]

        # ---------------- phase 1: qkv + rmsnorm + rope ----------------
        # section-major order [k0, q0, k1, q1, v]: pair-0 attention deps
        # complete early; v matmuls keep PE dense while the rope tail drains.
        # All Ln ops forced before all Exp ops -> exactly 2 ACT table loads.
        ln_insts, exp_insts = [], []
        with tc.tile_pool(name="ps1", bufs=1, space="PSUM") as ps1:
            sections = (
                    ("k", 0, bk, cos_k, sin_k, kt_),
                    ("q", 0, bq, cos_q, sin_q, qt),
                    ("k", 1, bk, cos_k, sin_k, kt_),
                    ("q", 1, bq, cos_q, sin_q, qt),
                    ("v", -1, None, None, None, None))
            for which, t, bias, cosT, sinT, dest in sections:
                if which == "v":
                    for sc in range(4):
                        for st in range(4):
                            pv = ps1.tile([128, HD], F32, tag="pv", bufs=2,
                                          name=f"pv{sc}_{st}")
                            for dt in range(NDT):
                                nc.tensor.matmul(
                                    pv[:, :],
                                    x_dt[dt][:, sc * 512 + st * 128: sc * 512 + (st + 1) * 128],
                                    w_dt[dt][:, 2 * HD:3 * HD],
                                    start=(dt == 0), stop=(dt == NDT - 1))
                            nc.vector.tensor_copy(vhat[sc][:, st, :, 0:64],
                                                  pv[:, :].rearrange("p (h d) -> p h d", h=HPC))
                    continue
                off = 0 if which == "q" else HD
                for sc in range(4):       # s-chunks of 512
                    s0 = sc * 512
                    pq = ps1.tile([128, 512], F32, tag="pq", bufs=2,
                                  name=f"pq{which}{t}_{sc}")
                    for dt in range(NDT):
                        nc.tensor.matmul(
                            pq[:, :],
                            w_dt[dt][:, off + t * 128: off + (t + 1) * 128],
                            x_dt[dt][:, s0:s0 + 512],
                            start=(dt == 0), stop=(dt == NDT - 1))
                    tt = tmp.tile([128, 512], F32, tag="tt", bufs=5, name=f"tt{which}{t}_{sc}")
                    nc.scalar.activation(tt[:, :], pq[:, :], AF.Identity,
                                         bias=bias[:, t:t + 1], scale=1.0)
                    sq = tmp.tile([128, 512], F32R, tag="sq", name=f"sq{which}{t}_{sc}")
                    nc.scalar.activation(sq[:, :], tt[:, :], AF.Square)
                    pss = ps1.tile([2, 512], F32, tag="pss", bufs=2,
                                   name=f"pss{which}{t}_{sc}")
                    nc.tensor.matmul(pss[:, :], onesblk[:, :], sq[:, :],
                                     start=True, stop=True)
                    # rsqrt(mean+eps) = Exp(-0.5*Ln(mean+eps)); both funcs in
                    # the natural_log_exp table set shared with softmax Exp
                    lns = tmp.tile([2, 512], F32, tag="lns", bufs=5, name=f"lns{which}{t}_{sc}")
                    li = nc.scalar.activation(lns[:, :], pss[:, :], AF.Ln,
                                              bias=eps_t[0:2, :], scale=1.0 / DH)
                    ln_insts.append(li)
                    rs = tmp.tile([2, 512], F32R, tag="rs", name=f"rs{which}{t}_{sc}")
                    ei = nc.scalar.activation(rs[:, :], lns[:, :], AF.Exp,
                                              scale=-0.5)
                    exp_insts.append(ei)
                    pb = ps1.tile([128, 512], F32, tag="pb",
                                  name=f"pb{which}{t}_{sc}")
                    nc.tensor.matmul(pb[:, :], ones2blk[:, :], rs[:, :],
                                     start=True, stop=True)
                    u = tmp.tile([128, 512], F32R, tag="u", name=f"u{which}{t}_{sc}")
                    nc.vector.tensor_mul(u[:, :], tt[:, :], pb[:, :])
                    psw = ps1.tile([128, 512], F32, tag="psw",
                                   name=f"psw{which}{t}_{sc}")
                    nc.tensor.matmul(psw[:, :], Pm[:, :], u[:, :],
                                     start=True, stop=True)
                    t1 = tmp.tile([128, 512], F32, tag="t1", name=f"t1{which}{t}_{sc}")
                    nc.vector.tensor_mul(t1[:, :], u[:, :].bitcast(F32),
                                         cosT[:, s0:s0 + 512])
                    t2 = tmp.tile([128, 512], F32, tag="t2", name=f"t2{which}{t}_{sc}")
                    nc.vector.tensor_mul(t2[:, :], psw[:, :], sinT[:, s0:s0 + 512])
                    nc.vector.tensor_add(dest[t][sc][:, :], t1[:, :], t2[:, :])

        for g in range(4):            # per-section groups (k0, q0, k1, q1)
            last_ln = ln_insts[g * 4 + 3]
            for ei in exp_insts[g * 4: g * 4 + 4]:
                tile.add_dep_helper(ei.ins, last_ln.ins, sync=False,
                                    reason="group ACT ln before exp (table sets)")

        # ---------------- phase 2: attention ----------------
        with tc.tile_pool(name="ps2", bufs=1, space="PSUM") as ps2:
            for pair in range(2):
                for qh in range(2):
                    q0 = qh * 1024
                    ps_sc = [ps2.tile([128, 1024], F32, tag=f"sc{h}",
                                      name=f"sc{pair}{qh}{h}") for h in range(2)]
                    ps_av = [[ps2.tile([65, 512], F32, tag=f"av{h}{qc}",
                                       name=f"av{pair}{qh}{h}{qc}")
                              for qc in range(2)] for h in range(2)]
                    for kt in range(16):
                        # scores: alternate heads so adjacent PE matmuls sit on
                        # different row groups (tile_position) and PSUM banks
                        for qc in range(2):
                            for h in range(2):
                                nc.tensor.matmul(
                                    ps_sc[h][:, qc * 512:(qc + 1) * 512],
                                    kt_[pair][kt // 4][h * 64:(h + 1) * 64,
                                                       (kt % 4) * 128:(kt % 4 + 1) * 128],
                                    qt[pair][qh * 2 + qc][h * 64:(h + 1) * 64, :],
                                    start=True, stop=True,
                                    tile_position=(h * 64, 0))
                        es = []
                        for h in range(2):
                            e = expp.tile([128, 1024], BF16, tag=f"e{h}",
                                          name=f"e{pair}{qh}{h}_{kt}")
                            nc.scalar.activation(e[:, :], ps_sc[h][:, :], AF.Exp,
                                                 scale=exp_scale)
                            es.append(e)
                        for h in range(2):
                            head = 2 * pair + h
                            for qc in range(2):
                                nc.tensor.matmul(
                                    ps_av[h][qc][:, :],
                                    vhat[kt // 4][:, kt % 4, head, :],
                                    es[h][:, qc * 512:(qc + 1) * 512],
                                    start=(kt == 0), stop=(kt == 15),
                                    skip_group_check=True)
                    # normalize: batch the 4 sumexp rows -> one reciprocal
                    # (rows live at 32-aligned partitions; rest memset to 1.0
                    # so the reciprocal stays finite and sel rows zero them)
                    se = tmp.tile([128, 512], F32, tag="se", name=f"se{pair}{qh}")
                    nc.vector.memset(se, 1.0)
                    for h in range(2):
                        for qc in range(2):
                            r0 = 32 * (2 * h + qc)
                            nc.vector.tensor_copy(se[r0:r0 + 1, :],
                                                  ps_av[h][qc][64:65, :])
                    recip4 = tmp.tile([128, 512], F32R, tag="recip4",
                                      name=f"rc{pair}{qh}")
                    from concourse.dve_ops import (RECIP_APPROX_FAST_CONSTS,
                                                   RECIPROCAL_APPROX_FAST)
                    _c = RECIP_APPROX_FAST_CONSTS
                    nc.vector._custom_dve(RECIPROCAL_APPROX_FAST,
                                          out=recip4[:, :], in0=se[:, :],
                                          s0=_c["s0"], s1=_c["s1"],
                                          imm2=_c["imm2"])
                    for qc in range(2):
                        col = q0 + qc * 512
                        avs2 = tmp.tile([128, 512], F32, tag="avs2",
                                        name=f"avs{pair}{qh}{qc}")
                        for h in range(2):
                            nc.vector.tensor_copy(avs2[h * 64:(h + 1) * 64, :],
                                                  ps_av[h][qc][0:64, :])
                        pb2 = ps2.tile([128, 512], F32, tag=f"av0{qc}",
                                       name=f"nb{pair}{qh}{qc}")
                        nc.tensor.matmul(pb2[:, :], sel[:, qc, :], recip4[:, :],
                                         start=True, stop=True)
                        nc.vector.tensor_mul(
                            vmix[pair][qh][:, qc * 512:(qc + 1) * 512],
                            avs2[:, :], pb2[:, :])

        # ---------------- phase 3: out proj ----------------
        with tc.tile_pool(name="ps3", bufs=1, space="PSUM") as ps3:
            for st in range(16):
                for n in range(2):
                    po = ps3.tile([128, 512], F32, tag="po", bufs=3,
                                  name=f"po{st}_{n}")
                    for t in range(2):
                        nc.tensor.matmul(
                            po[:, :],
                            vmix[t][st // 8][:, (st % 8) * 128:(st % 8 + 1) * 128],
                            wout[:, t, n * 512:(n + 1) * 512],
                            start=(t == 0), stop=(t == 1))
                    o = outp.tile([128, 512], BF16, tag="o", name=f"o{st}_{n}")
                    if (st * 2 + n) % 2 == 0:
                        nc.scalar.activation(o[:, :], po[:, :], AF.Copy)
                    else:
                        nc.vector.tensor_copy(o[:, :], po[:, :])
                    nc.sync.dma_start(
                        out=out_d.ap()[st * 128:(st + 1) * 128,
                                       n * 512:(n + 1) * 512],
                        in_=o[:, :])

    nc.compile()
    return nc


def host_prep(x, pos, Wqkv, bqkv, Wout, bout, q_scale, k_scale):
    """Build per-core input maps + shared-table decision."""
    x = np.asarray(x, dtype=np.float32)
    pos = np.asarray(pos, dtype=np.float32).reshape(-1)
    Wqkv = np.asarray(Wqkv, dtype=np.float32)
    bqkv = np.asarray(bqkv, dtype=np.float32)
    Wout = np.asarray(Wout, dtype=np.float32)
    q_scale = np.asarray(q_scale, dtype=np.float32)
    k_scale = np.asarray(k_scale, dtype=np.float32)

    shared = bool(np.array_equal(q_scale, k_scale))
    exp_scale = (1.0 / np.sqrt(DH)) if shared else 1.0

    # rope base tables [128, S]
    i_of_p = (np.arange(128) % 64) // 2            # pair index
    sign = np.where(np.arange(128) % 2 == 0, 1.0, -1.0)
    omega = THETA ** (-np.arange(0, DH, 2, dtype=np.float64) / DH)  # [32]
    ang = pos[None, :].astype(np.float64) * omega[:, None]          # [32, S]
    cosb = np.cos(ang)[i_of_p, :]                  # [128, S]
    sinb = np.sin(ang)[i_of_p, :] * sign[:, None]

    def tables(scale_vec, extra):
        sv = np.tile(scale_vec, 2)                 # [128]
        svx = np.tile(scale_vec[np.arange(64) ^ 1], 2)
        cosT = (cosb * sv[:, None] * extra).astype(np.float32)
        sinT = (sinb * svx[:, None] * extra).astype(np.float32)
        return np.ascontiguousarray(cosT), np.ascontiguousarray(sinT)

    cos_k, sin_k = tables(k_scale, 1.0)
    if not shared:
        cos_q, sin_q = tables(q_scale, 1.0 / np.sqrt(DH))

    Pm = np.zeros((128, 128), dtype=np.float32)
    Pm[np.arange(128), np.arange(128) ^ 1] = 1.0
    onesblk = np.zeros((128, 2), dtype=np.float32)
    onesblk[0:64, 0] = 1.0
    onesblk[64:128, 1] = 1.0
    ones2blk = np.zeros((2, 128), dtype=np.float32)
    ones2blk[0, 0:64] = 1.0
    ones2blk[1, 64:128] = 1.0
    # sel[qc]: [4, 128] selecting reciprocal row (h, qc) for partitions h*64..
    sel = np.zeros((128, 2, 128), dtype=np.float32)
    for qc in range(2):
        for h in range(2):
            sel[32 * (2 * h + qc), qc, h * 64:(h + 1) * 64] = 1.0

    bf = ml_dtypes.bfloat16
    in_maps = []
    for c in range(NC):
        b, g = c // 4, c % 4
        xT = np.ascontiguousarray(
            x[b].T.reshape(NDT, 128, S).transpose(1, 0, 2)).astype(bf)
        wq = Wqkv[:, g * HD:(g + 1) * HD]
        wk = Wqkv[:, DM + g * HD: DM + (g + 1) * HD]
        wv = Wqkv[:, 2 * DM + g * HD: 2 * DM + (g + 1) * HD]
        w_all = np.ascontiguousarray(
            np.concatenate([wq, wk, wv], axis=1)
            .reshape(NDT, 128, 3 * HD).transpose(1, 0, 2)).astype(bf)
        wo = np.ascontiguousarray(
            Wout[g * HD:(g + 1) * HD, :]
            .reshape(2, 128, DM).transpose(1, 0, 2)).astype(bf)
        bqs = np.ascontiguousarray(
            bqkv[g * HD:(g + 1) * HD].reshape(2, 128).T)         # [128, 2]
        bks = np.ascontiguousarray(
            bqkv[DM + g * HD: DM + (g + 1) * HD].reshape(2, 128).T)
        m = {"xT": xT, "w_all": w_all, "wout": wo, "bq": bqs, "bk": bks,
             "cos_k": cos_k, "sin_k": sin_k, "Pswap": Pm, "onesblk": onesblk,
             "ones2blk": ones2blk, "sel": sel}
        if not shared:
            m["cos_q"] = cos_q
            m["sin_q"] = sin_q
        in_maps.append(m)

    bias_row = (bqkv[2 * DM:] @ Wout + np.asarray(bout, dtype=np.float32)) \
        .astype(np.float32)                                       # [1024]
    return in_maps, shared, float(exp_scale), bias_row


def _install_ntff_shim():
    """Make trace=True usable: this image lacks antenv.axon_hooks; recreate
    it against the baked libaxon_pjrt.so C ABI (no-op if already present)."""
    try:
        from antenv.axon_hooks import get_axon_ntff_profile_hook  # noqa: F401
        return
    except ImportError:
        pass
    try:
        import types, ctypes, contextlib
        import antenv
        lib = ctypes.CDLL("/opt/axon/libaxon_pjrt.so")
        if not hasattr(lib, "axon_start_nrt_profile"):
            raise OSError("no profile symbols")
        lib.axon_start_nrt_profile.argtypes = [ctypes.POINTER(ctypes.c_int64),
                                               ctypes.c_size_t]
        lib.axon_start_nrt_profile.restype = ctypes.c_int64
        lib.axon_stop_nrt_profile.argtypes = [ctypes.c_char_p]
        lib.axon_stop_nrt_profile.restype = ctypes.c_int64

        @contextlib.contextmanager
        def _hook(output_dir, device_ids):
            import jax
            jax.devices()
            if device_ids:
                ids = (ctypes.c_int64 * len(device_ids))(*device_ids)
                rc = lib.axon_start_nrt_profile(ids, len(device_ids))
            else:
                rc = lib.axon_start_nrt_profile(None, 0)
            if rc != 0:
                raise RuntimeError(f"axon_start_nrt_profile rc={rc}")
            try:
                yield
            finally:
                lib.axon_stop_nrt_profile(str(output_dir).encode())

        mod = types.ModuleType("antenv.axon_hooks")
        mod.get_axon_ntff_profile_hook = lambda: _hook
        mod.set_axon_ntff_profile_hook = lambda h: None
        sys.modules["antenv.axon_hooks"] = mod
        antenv.axon_hooks = mod
    except Exception:
        os.environ["BASS_NEVER_TRACE"] = "1"   # degrade: run untraced


def kernel(x, pos, Wqkv, bqkv, Wout, bout, q_scale, k_scale):
    global LAST_RESULTS
    if os.environ.get("BASS_TRACE"):
        _install_ntff_shim()
    in_maps, shared, exp_scale, bias_row = host_prep(
        x, pos, Wqkv, bqkv, Wout, bout, q_scale, k_scale)

    key = (shared, round(exp_scale, 9))
    if key not in _CACHED:
        _CACHED[key] = build_program(exp_scale, shared)
    nc = _CACHED[key]

    res = bass_utils.run_bass_kernel_spmd(
        nc, in_maps, list(range(NC)),
        trace=bool(os.environ.get("BASS_TRACE")))
    LAST_RESULTS = res

    out = np.empty((B, S, DM), dtype=np.float32)
    for b in range(B):
        acc = bias_row[None, :].astype(np.float32).repeat(S, axis=0)
        for g in range(4):
            acc = acc + res.results[b * 4 + g]["outp"].astype(np.float32)
        out[b] = acc
    return out
